# revision 1
# baseline (speedup 1.0000x reference)
"""GatedCRFLoss Trainium2 kernel: 8-core SPMD over (B,H) row stripes.

Device computes, per core (128 rows of one image):
  - softmax(logit) + cross-entropy partial sums (once)
  - for each offset pair (dx, dy)/( -dx,-dy): numerator partial sums of the
    CRF energy, accumulated per-partition into an f32 accumulator column
Host computes: denominators (exact rectangle sums of dst), final reductions.
"""
import sys

sys.path.insert(0, "/opt/trn_rl_repo")

import numpy as np
import ml_dtypes

SPAN = 11
B, C, H, W = 4, 3, 256, 512
NCORES = 8
RPC = 128  # rows per core
HALO = SPAN
SIG_RGB = 0.1
SIG_XY = 6.0
SIG_DEPTH = 0.2
SC_RGB = -0.5 / (SIG_RGB * SIG_RGB)   # -50
SC_DEP = -0.5 / (SIG_DEPTH * SIG_DEPTH)  # -12.5

DYS = list(range(-SPAN, 0)) + list(range(1, SPAN + 1))
PAIRS = [(dx, dy) for dx in range(1, SPAN + 1) for dy in DYS]
NPAIR = len(PAIRS)  # 242
COL_CE_LDS = 2 * NPAIR      # 484: sum(L_ce * dst)
COL_CE_L = 2 * NPAIR + 1    # 485: sum(L_ce)
ACC_W = 512

SUB_ENG = "gpsimd"   # engine for the big 4-channel subtract
SQ_ENG = "scalar"    # engine for the square

BF = None  # set lazily (mybir.dt.bfloat16)


def _build(dx_list):
    import concourse.bass as bass  # noqa: F401
    import concourse.tile as tile
    from concourse import bacc, mybir

    global BF
    BF = mybir.dt.bfloat16
    F32 = mybir.dt.float32
    Alu = mybir.AluOpType
    Act = mybir.ActivationFunctionType

    nc = bacc.Bacc("TRN2", target_bir_lowering=False, debug=False,
                   num_devices=NCORES)

    img_d = nc.dram_tensor("img4", [RPC + HALO, 4 * W], BF, kind="ExternalInput").ap()
    lg_d = nc.dram_tensor("lg", [RPC + HALO, 3 * W], F32, kind="ExternalInput").ap()
    ds_d = nc.dram_tensor("ds", [RPC + HALO, W], F32, kind="ExternalInput").ap()
    tgt_d = nc.dram_tensor("tgt", [RPC, W], F32, kind="ExternalInput").ap()
    rm_d = nc.dram_tensor("rowmask", [RPC, 16], F32, kind="ExternalInput").ap()
    bt_d = nc.dram_tensor("biastbl", [RPC, 256], F32, kind="ExternalInput").ap()
    out_d = nc.dram_tensor("out", [RPC, ACC_W], F32, kind="ExternalOutput").ap()
    prb_d = nc.dram_tensor("prb_scratch", [RPC + HALO, 4 * W], BF).ap()

    img3 = img_d.rearrange("r (c w) -> r c w", w=W)
    lg3 = lg_d.rearrange("r (c w) -> r c w", w=W)
    prb3 = prb_d.rearrange("r (c w) -> r c w", w=W)

    sub_eng = {"gpsimd": nc.gpsimd, "vector": nc.vector}[SUB_ENG]

    with tile.TileContext(nc) as tc:
        from contextlib import ExitStack
        with ExitStack() as ctx:
            cp = ctx.enter_context(tc.tile_pool(name="const", bufs=1))
            vp = ctx.enter_context(tc.tile_pool(name="vshift", bufs=2))
            tp = ctx.enter_context(tc.tile_pool(name="tmp", bufs=3))

            IMGu = cp.tile([RPC, 4, W], BF, tag="IMGu")
            IMGuo = cp.tile([RPC, 4, W], BF, tag="IMGuo")
            PRBu = cp.tile([RPC, 4, W], BF, tag="PRBu")
            PRBuo = cp.tile([RPC, 4, W], BF, tag="PRBuo")
            PRBH = cp.tile([HALO, 4, W], BF, tag="PRBH")
            LG = cp.tile([RPC, 3, W], F32, tag="LG")
            LGH = cp.tile([HALO, 3, W], F32, tag="LGH")
            TGT = cp.tile([RPC, W], F32, tag="TGT")
            DSF = cp.tile([RPC, W], F32, tag="DSF")
            DSFH = cp.tile([HALO, W], F32, tag="DSFH")
            RM = cp.tile([RPC, 16], F32, tag="RM")
            BT = cp.tile([RPC, 256], F32, tag="BT")
            ACC = cp.tile([RPC, ACC_W], F32, tag="ACC")

            nc.sync.dma_start(IMGu[:, :, :], img3[HALO:, :, :])
            nc.sync.dma_start(IMGuo[:, :, 0:W - 1], img3[HALO:, :, 1:W])
            nc.sync.dma_start(LG[:, :, :], lg3[HALO:, :, :])
            nc.sync.dma_start(LGH[:, :, :], lg3[0:HALO, :, :])
            nc.sync.dma_start(TGT[:, :], tgt_d[:, :])
            nc.sync.dma_start(DSF[:, :], ds_d[HALO:, :])
            nc.sync.dma_start(DSFH[:, :], ds_d[0:HALO, :])
            nc.sync.dma_start(RM[:, :], rm_d[:, :])
            nc.sync.dma_start(BT[:, :], bt_d[:, :])

            nc.gpsimd.memset(ACC[:, :], 0.0)

            # ---- softmax (main stripe) ----
            EXL = cp.tile([RPC, 3, W], F32, tag="EXL")
            SS = cp.tile([RPC, W], F32, tag="SS")
            RR = cp.tile([RPC, W], F32, tag="RR")
            nc.scalar.activation(EXL[:, :, :], LG[:, :, :], Act.Exp)
            nc.vector.tensor_add(SS[:, :], EXL[:, 0, :], EXL[:, 1, :])
            nc.vector.tensor_tensor(SS[:, :], SS[:, :], EXL[:, 2, :], Alu.add)
            nc.vector.reciprocal(RR[:, :], SS[:, :])
            for c in range(3):
                nc.vector.tensor_mul(PRBu[:, c, :], EXL[:, c, :], RR[:, :])
            nc.vector.tensor_copy(PRBu[:, 3, :], DSF[:, :])
            # odd-shifted copies for bf16 alignment
            nc.vector.tensor_copy(PRBuo[:, :, 0:W - 1], PRBu[:, :, 1:W])

            # ---- softmax (halo rows) ----
            EXLH = cp.tile([HALO, 3, W], F32, tag="EXLH")
            SSH = cp.tile([HALO, W], F32, tag="SSH")
            RRH = cp.tile([HALO, W], F32, tag="RRH")
            nc.scalar.activation(EXLH[:, :, :], LGH[:, :, :], Act.Exp)
            nc.vector.tensor_add(SSH[:, :], EXLH[:, 0, :], EXLH[:, 1, :])
            nc.vector.tensor_tensor(SSH[:, :], SSH[:, :], EXLH[:, 2, :], Alu.add)
            nc.vector.reciprocal(RRH[:, :], SSH[:, :])
            for c in range(3):
                nc.vector.tensor_mul(PRBH[:, c, :], EXLH[:, c, :], RRH[:, :])
            nc.vector.tensor_copy(PRBH[:, 3, :], DSFH[:, :])

            nc.sync.dma_start(prb3[HALO:, :, :], PRBu[:, :, :])
            nc.sync.dma_start(prb3[0:HALO, :, :], PRBH[:, :, :])

            # ---- cross entropy partials ----
            LS = cp.tile([RPC, W], F32, tag="LS")
            M1 = cp.tile([RPC, W], F32, tag="M1")
            M2 = cp.tile([RPC, W], F32, tag="M2")
            D10 = cp.tile([RPC, W], F32, tag="D10")
            D21 = cp.tile([RPC, W], F32, tag="D21")
            T1 = cp.tile([RPC, W], F32, tag="T1")
            LT = cp.tile([RPC, W], F32, tag="LT")
            LCE = cp.tile([RPC, W], F32, tag="LCE")
            CES = cp.tile([RPC, W], F32, tag="CES")
            nc.scalar.activation(LS[:, :], SS[:, :], Act.Ln)
            nc.vector.tensor_scalar(M1[:, :], TGT[:, :], 0.5, None, Alu.is_ge)
            nc.vector.tensor_scalar(M2[:, :], TGT[:, :], 1.5, None, Alu.is_ge)
            nc.vector.tensor_sub(D10[:, :], LG[:, 1, :], LG[:, 0, :])
            nc.vector.tensor_sub(D21[:, :], LG[:, 2, :], LG[:, 1, :])
            nc.vector.tensor_mul(T1[:, :], M1[:, :], D10[:, :])
            nc.vector.tensor_add(LT[:, :], LG[:, 0, :], T1[:, :])
            nc.vector.tensor_mul(T1[:, :], M2[:, :], D21[:, :])
            nc.vector.tensor_tensor(LT[:, :], LT[:, :], T1[:, :], Alu.add)
            nc.vector.tensor_sub(LCE[:, :], LS[:, :], LT[:, :])
            nc.vector.scalar_tensor_tensor(
                CES[:, :], LCE[:, :], 1.0, DSF[:, :],
                Alu.mult, Alu.mult,
                accum_out=ACC[:, COL_CE_LDS:COL_CE_LDS + 1])
            nc.vector.tensor_reduce(
                ACC[:, COL_CE_L:COL_CE_L + 1], LCE[:, :],
                mybir.AxisListType.X, Alu.add)

            # ---- pair loop ----
            for dx in dx_list:
                IMGv = vp.tile([RPC, 4, W], BF, tag="IMGv")
                IMGvo = vp.tile([RPC, 4, W], BF, tag="IMGvo")
                PRBv = vp.tile([RPC, 4, W], BF, tag="PRBv")
                PRBvo = vp.tile([RPC, 4, W], BF, tag="PRBvo")
                DSM = vp.tile([RPC, W], BF, tag="DSM")
                DSMo = vp.tile([RPC, W], BF, tag="DSMo")
                nc.sync.dma_start(IMGv[:, :, :], img3[HALO - dx:HALO - dx + RPC, :, :])
                nc.sync.dma_start(IMGvo[:, :, 0:W - 1],
                                  img3[HALO - dx:HALO - dx + RPC, :, 1:W])
                nc.sync.dma_start(PRBv[:, :, :], prb3[HALO - dx:HALO - dx + RPC, :, :])
                nc.sync.dma_start(PRBvo[:, :, 0:W - 1],
                                  prb3[HALO - dx:HALO - dx + RPC, :, 1:W])
                nc.vector.tensor_scalar(DSM[:, :], PRBu[:, 3, :],
                                        RM[:, dx:dx + 1], None, Alu.mult)
                nc.vector.tensor_scalar(DSMo[:, 0:W - 1], PRBuo[:, 3, 0:W - 1],
                                        RM[:, dx:dx + 1], None, Alu.mult)

                for dy in DYS:
                    k = PAIRS.index((dx, dy))
                    ady = abs(dy)
                    FD = W - ady
                    if dy > 0:
                        if dy % 2 == 0:
                            uimg, uprb, udsm, us = IMGu, PRBu, DSM, dy
                        else:
                            uimg, uprb, udsm, us = IMGuo, PRBuo, DSMo, dy - 1
                        vimg, vprb, vs = IMGv, PRBv, 0
                    else:
                        uimg, uprb, udsm, us = IMGu, PRBu, DSM, 0
                        if ady % 2 == 0:
                            vimg, vprb, vs = IMGv, PRBv, ady
                        else:
                            vimg, vprb, vs = IMGvo, PRBvo, ady - 1

                    X4 = tp.tile([RPC, 4, W], BF, tag="X4")
                    sub_eng.tensor_tensor(
                        X4[:, :, 0:FD], uimg[:, :, us:us + FD],
                        vimg[:, :, vs:vs + FD], Alu.subtract)
                    Y4 = tp.tile([RPC, 4, W], BF, tag="Y4")
                    if SQ_ENG == "scalar":
                        nc.scalar.activation(Y4[:, :, 0:FD], X4[:, :, 0:FD],
                                             Act.Square)
                    else:
                        nc.vector.tensor_mul(Y4[:, :, 0:FD], X4[:, :, 0:FD],
                                             X4[:, :, 0:FD])
                    S = tp.tile([RPC, W], BF, tag="S")
                    nc.vector.tensor_add(S[:, 0:FD], Y4[:, 0, 0:FD], Y4[:, 1, 0:FD])
                    nc.vector.tensor_tensor(S[:, 0:FD], S[:, 0:FD], Y4[:, 2, 0:FD],
                                            Alu.add)
                    E1 = tp.tile([RPC, W], BF, tag="E1")
                    nc.scalar.activation(E1[:, 0:FD], S[:, 0:FD], Act.Exp,
                                         bias=BT[:, k:k + 1], scale=SC_RGB)
                    E2 = tp.tile([RPC, W], BF, tag="E2")
                    nc.scalar.activation(E2[:, 0:FD], Y4[:, 3, 0:FD], Act.Exp,
                                         scale=SC_DEP)
                    M3 = tp.tile([RPC, 3, W], BF, tag="M3")
                    nc.vector.tensor_tensor(
                        M3[:, :, 0:FD], uprb[:, 0:3, us:us + FD],
                        vprb[:, 0:3, vs:vs + FD], Alu.mult)
                    P = tp.tile([RPC, W], BF, tag="P")
                    nc.vector.tensor_add(P[:, 0:FD], M3[:, 0, 0:FD], M3[:, 1, 0:FD])
                    nc.vector.tensor_tensor(P[:, 0:FD], P[:, 0:FD], M3[:, 2, 0:FD],
                                            Alu.add)
                    K = tp.tile([RPC, W], BF, tag="K")
                    nc.vector.tensor_add(K[:, 0:FD], E1[:, 0:FD], E2[:, 0:FD])
                    AN = tp.tile([RPC, W], BF, tag="AN")
                    nc.vector.scalar_tensor_tensor(
                        AN[:, 0:FD], P[:, 0:FD], 1.0, K[:, 0:FD],
                        Alu.subtract, Alu.mult)
                    ZS1 = tp.tile([RPC, W], BF, tag="ZS1")
                    nc.vector.scalar_tensor_tensor(
                        ZS1[:, 0:FD], AN[:, 0:FD], -1.0, vprb[:, 3, vs:vs + FD],
                        Alu.mult, Alu.mult, accum_out=ACC[:, k:k + 1])
                    ZS2 = tp.tile([RPC, W], BF, tag="ZS2")
                    nc.vector.scalar_tensor_tensor(
                        ZS2[:, 0:FD], AN[:, 0:FD], -1.0, udsm[:, us:us + FD],
                        Alu.mult, Alu.mult,
                        accum_out=ACC[:, NPAIR + k:NPAIR + k + 1])

            nc.sync.dma_start(out_d[:, :], ACC[:, :])

    nc.compile()
    return nc


_CACHE = {}


def _get_nc(dx_list):
    key = tuple(dx_list)
    if key not in _CACHE:
        _CACHE[key] = _build(dx_list)
    return _CACHE[key]


def _make_inputs(logit, target, image, depth, destination_map):
    """Build the 8 per-core input dicts."""
    bf = ml_dtypes.bfloat16
    in_maps = []
    bias = np.zeros((RPC, 256), np.float32)
    for k, (dx, dy) in enumerate(PAIRS):
        bias[:, k] = -0.5 * (dx * dx + dy * dy) / (SIG_XY * SIG_XY)
    for cidx in range(NCORES):
        b = cidx // 2
        r0 = RPC * (cidx % 2)
        rows = np.arange(r0 - HALO, r0 + RPC)
        valid = rows >= 0
        rv = np.clip(rows, 0, H - 1)

        def stripe(x2d, zero_invalid=True):
            # x2d: (H, W) -> (HALO+RPC, W) with out-of-image rows zeroed
            s = x2d[rv].astype(np.float32)
            if zero_invalid:
                s[~valid] = 0.0
            return s

        img4 = np.zeros((RPC + HALO, 4, W), np.float32)
        for c in range(3):
            img4[:, c, :] = stripe(np.asarray(image[b, c]))
        img4[:, 3, :] = stripe(np.asarray(depth[b, 0]))
        lg = np.zeros((RPC + HALO, 3, W), np.float32)
        for c in range(3):
            lg[:, c, :] = stripe(np.asarray(logit[b, c]))
        ds = stripe(np.asarray(destination_map[b, 0]))
        tgt = np.asarray(target[b, r0:r0 + RPC]).astype(np.float32)
        rowmask = np.ones((RPC, 16), np.float32)
        if r0 == 0:
            for dx in range(1, SPAN + 1):
                rowmask[:dx, dx] = 0.0
        in_maps.append({
            "img4": img4.reshape(RPC + HALO, 4 * W).astype(bf),
            "lg": lg.reshape(RPC + HALO, 3 * W),
            "ds": ds,
            "tgt": tgt,
            "rowmask": rowmask,
            "biastbl": bias,
        })
    return in_maps


def _dens(destination_map):
    """Exact denominators per offset via integral image (f64)."""
    d = np.asarray(destination_map[:, 0]).astype(np.float64).sum(axis=0)
    ii = np.zeros((H + 1, W + 1))
    ii[1:, 1:] = d.cumsum(0).cumsum(1)

    def rect(r0, r1, c0, c1):
        return ii[r1, c1] - ii[r0, c1] - ii[r1, c0] + ii[r0, c0]

    den_pos = np.zeros(NPAIR)
    den_neg = np.zeros(NPAIR)
    for k, (dx, dy) in enumerate(PAIRS):
        if dy > 0:
            den_pos[k] = rect(0, H - dx, 0, W - dy)
            den_neg[k] = rect(dx, H, dy, W)
        else:
            den_pos[k] = rect(0, H - dx, -dy, W)
            den_neg[k] = rect(dx, H, 0, W + dy)
    return den_pos, den_neg


def _run(inputs, dx_list, trace=False):
    from concourse.bass_utils import run_bass_kernel_spmd
    nc = _get_nc(dx_list)
    in_maps = _make_inputs(inputs["logit"], inputs["target"], inputs["image"],
                           inputs["depth"], inputs["destination_map"])
    res = run_bass_kernel_spmd(nc, in_maps, core_ids=list(range(NCORES)),
                               trace=trace)
    outs = np.stack([np.asarray(res.results[i]["out"], np.float64)
                     for i in range(NCORES)])  # (8, 128, 512)
    return outs, res


def _post(outs, destination_map):
    tot = outs.sum(axis=(0, 1))  # (512,)
    num_pos = tot[:NPAIR]
    num_neg = tot[NPAIR:2 * NPAIR]
    den_pos, den_neg = _dens(destination_map)
    e_sum = (num_pos / den_pos).sum() + (num_neg / den_neg).sum()
    K2 = (2 * SPAN + 1) ** 2
    l_gcrf = e_sum / K2

    n = B * H * W
    sum_lds = tot[COL_CE_LDS]
    sum_l = tot[COL_CE_L]
    l1 = sum_lds / n
    l2 = (sum_l - sum_lds) / n
    count = float(np.asarray(destination_map, np.float64).mean())
    ce = l1 * (1.0 - count) + l2 * count
    return np.float32(ce), np.float32(l_gcrf)


def kernel(logit, target, image, depth, destination_map, source_map):
    inputs = dict(logit=logit, target=target, image=image, depth=depth,
                  destination_map=destination_map)
    outs, _ = _run(inputs, list(range(1, SPAN + 1)))
    return _post(outs, destination_map)

